# revision 27
# baseline (speedup 1.0000x reference)
"""EdgeCNN (DGCNN) Bass/Tile kernel for TRN2 — one batch element per core.

Per edge-conv layer (N=1024 points, K=20 neighbors):
  1. PE fp32r: packed-key matmul  pd[n,j] = 2<xn,xj> - S[j]   (PSUM, fp32)
     (the -S[n] row term is dropped: constant per row, cannot change top-k)
  2. ACT: PSUM -> SBUF copy rounded to f32r (zeroes low mantissa bits)
  3. DVE: OR in column index j -> packed keys; 3x max8 + 2x match_replace
     -> top-20 packed keys; extract j
  4. idx -> DRAM -> read back in dma_gather wrapped layout (partition = n%16)
  5. SWDGE dma_gather (per tile, round-robin over 4 SWDGE queues) of rows of
     a = x @ (g~ Wn)^T; DVE strided reduce_max over k
  6. PE: c-matmul fp32r (c = x @ (g~(Wc-Wn))^T + b) + transpose(m) in PSUM
  7. ACT: leaky-relu (Prelu alpha=0.2) PSUM -> next layer xT
Head: conv5 via K-chunk accumulation (fp32r), global max-pool, 3 FC layers.
"""

import contextlib

import numpy as np

import concourse.bass as bass
import concourse.bacc as bacc
import concourse.mybir as mybir
from concourse.tile import TileContext
from concourse.masks import make_identity

F32 = mybir.dt.float32
F32R = mybir.dt.float32r
U32 = mybir.dt.uint32
I16 = mybir.dt.int16
F16 = mybir.dt.float16
AF = mybir.ActivationFunctionType
ALU = mybir.AluOpType
AX = mybir.AxisListType

N = 1024
KNN = 20
NT = 8
NEG_SLOPE = 0.2
BNI = np.float32(1.0 / np.sqrt(1.0 + 1e-5))
LAYERS = [(3, 64), (64, 64), (64, 128), (128, 256)]
NEG_BIG = -3.0e38


def host_prep(inp):
    """Fold BN scale/bias into weights; transpose for device layout."""
    d = {}
    for li, (C, O) in enumerate(LAYERS, start=1):
        W = inp[f'W{li}'].astype(np.float32)
        g = inp[f'g{li}'].astype(np.float32)
        b = inp[f'b{li}'].astype(np.float32)
        gt = g * BNI
        Wn = W[:, :C]
        Wc = W[:, C:]
        d[f'wnt{li}'] = np.ascontiguousarray((gt[:, None] * Wn).T)          # (C, O)
        d[f'wdt{li}'] = np.ascontiguousarray((gt[:, None] * (Wc - Wn)).T)   # (C, O)
        d[f'bs{li}'] = b.reshape(1, O).copy()
    g5 = inp['g5'].astype(np.float32) * BNI
    d['w5t'] = np.ascontiguousarray((g5[:, None] * inp['W5']).T)            # (512, 512)
    d['b5'] = inp['b5'].reshape(1, 512).astype(np.float32).copy()
    g1 = inp['bng1'].astype(np.float32) * BNI
    d['wfc1'] = np.ascontiguousarray((g1[:, None] * inp['fc1_w']).T)        # (512, 256)
    bf1 = g1 * inp['fc1_b'].astype(np.float32) + inp['bnb1'].astype(np.float32)
    d['bfc1'] = np.ascontiguousarray(bf1.reshape(2, 128).T)                 # (128, 2)
    g2 = inp['bng2'].astype(np.float32) * BNI
    d['wfc2'] = np.ascontiguousarray((g2[:, None] * inp['fc2_w']).T)        # (256, 128)
    bf2 = g2 * inp['fc2_b'].astype(np.float32) + inp['bnb2'].astype(np.float32)
    d['bfc2'] = np.ascontiguousarray(bf2.reshape(128, 1))                   # (128, 1)
    d['wfc3'] = np.ascontiguousarray(inp['fc3_w'].T)                        # (128, 40)
    d['bfc3'] = inp['fc3_b'].reshape(1, 40).astype(np.float32).copy()
    return d


def build_nc():
    nc = bacc.Bacc("TRN2", target_bir_lowering=False, debug=False, num_devices=8,
                   num_swdge_queues=4)
    with TileContext(nc) as tc:
        _trace(nc, tc)
    nc.compile()
    return nc


def _slice_cols(ap, cols):
    return bass.AP(ap.tensor, ap.offset + cols.start,
                   [ap.ap[0], [1, cols.stop - cols.start]])


def _trace(nc, tc):
    with contextlib.ExitStack() as ctx:
        dram = ctx.enter_context(tc.tile_pool(name="dram", bufs=1, space="DRAM"))
        consts = ctx.enter_context(tc.tile_pool(name="consts", bufs=1))
        persist = ctx.enter_context(tc.tile_pool(name="persist", bufs=1))
        sb = ctx.enter_context(tc.tile_pool(name="sb", bufs=2))
        keyp = ctx.enter_context(tc.tile_pool(name="keyp", bufs=2))
        gath = ctx.enter_context(tc.tile_pool(name="gath", bufs=2))
        mp = ctx.enter_context(tc.tile_pool(name="mp", bufs=1))
        psb = ctx.enter_context(tc.tile_pool(name="psb", bufs=1, space="PSUM"))
        pxp = ctx.enter_context(tc.tile_pool(name="pxp", bufs=2, space="PSUM"))
        pss = ctx.enter_context(tc.tile_pool(name="pss", bufs=2, space="PSUM"))

        # ---- DRAM I/O ----
        x_d = dram.tile([N, 3], F32, kind="ExternalInput", uniquify=False, name="x")
        win = {}
        for li, (C, O) in enumerate(LAYERS, start=1):
            win[f'wnt{li}'] = dram.tile([C, O], F32, kind="ExternalInput", uniquify=False, name=f"wnt{li}")
            win[f'wdt{li}'] = dram.tile([C, O], F32, kind="ExternalInput", uniquify=False, name=f"wdt{li}")
            win[f'bs{li}'] = dram.tile([1, O], F32, kind="ExternalInput", uniquify=False, name=f"bs{li}")
        w5t_d = dram.tile([512, 512], F32, kind="ExternalInput", uniquify=False, name="w5t")
        b5_d = dram.tile([1, 512], F32, kind="ExternalInput", uniquify=False, name="b5")
        wfc1_d = dram.tile([512, 256], F32, kind="ExternalInput", uniquify=False, name="wfc1")
        bfc1_d = dram.tile([128, 2], F32, kind="ExternalInput", uniquify=False, name="bfc1")
        wfc2_d = dram.tile([256, 128], F32, kind="ExternalInput", uniquify=False, name="wfc2")
        bfc2_d = dram.tile([128, 1], F32, kind="ExternalInput", uniquify=False, name="bfc2")
        wfc3_d = dram.tile([128, 40], F32, kind="ExternalInput", uniquify=False, name="wfc3")
        bfc3_d = dram.tile([1, 40], F32, kind="ExternalInput", uniquify=False, name="bfc3")
        out_d = dram.tile([40, 1], F32, kind="ExternalOutput", uniquify=False, name="out")

        a_ds = {li: dram.tile([N, O], F16 if li >= 3 else F32, name=f"a_d{li}")
                for li, (C, O) in enumerate(LAYERS, start=1)}
        jw_ds = {li: dram.tile([NT * (KNN * 8), 128], I16, name=f"jw_d{li}")
                 for li in range(1, 5)}

        # ---- consts ----
        iotaJ = consts.tile([128, N], U32, tag="iotaJ")
        nc.gpsimd.iota(iotaJ[:, :], [[1, N]], base=0, channel_multiplier=0)
        ident = consts.tile([128, 128], F32, tag="ident")
        make_identity(nc, ident[:, :])
        onescol = consts.tile([128, 1], F32, tag="onescol")
        nc.vector.memset(onescol[:, :], 1.0)
        onesrow = consts.tile([1, N], F32, tag="onesrow")
        nc.vector.memset(onesrow[:, :], 1.0)
        onesrow_r = consts.tile([1, N], F32R, tag="onesrow_r")
        nc.scalar.activation(onesrow_r[:, :], onesrow[:, :], AF.Copy)

        # persistent feature tensors (F32 + F32R copies)
        x0T = persist.tile([3, N], F32, tag="x0T")
        x0r = persist.tile([3, N], F32R, tag="x0r")
        x1T = persist.tile([64, N], F32, tag="x1T")
        x1r = persist.tile([64, N], F32R, tag="x1r")
        x2T = persist.tile([64, N], F32, tag="x2T")
        x2r = persist.tile([64, N], F32R, tag="x2r")
        x3T = persist.tile([128, N], F32, tag="x3T")
        x3r = persist.tile([128, N], F32R, tag="x3r")
        x4Ta = persist.tile([128, N], F32, tag="x4Ta")
        x4ra = persist.tile([128, N], F32R, tag="x4ra")
        x4Tb = persist.tile([128, N], F32, tag="x4Tb")
        x4rb = persist.tile([128, N], F32R, tag="x4rb")

        # load x transposed: x_d is (N, 3) row-major
        xap = x_d[:, :]
        nc.sync.dma_start(
            x0T[:, :], bass.AP(xap.tensor, xap.offset, [[1, 3], [3, N]]))
        nc.scalar.activation(x0r[:, :], x0T[:, :], AF.Copy)

        def edge_layer(li, xT, xr, C, O, out_parts, pre_tiles=None):
            """out_parts: list of (xT_part_ap, xr_part_ap, orow)."""
            a_d = a_ds[li]
            jw_d = jw_ds[li]
            adt = F16 if li >= 3 else F32
            wnt = sb.tile([C, O], F32, tag="wnt")
            wdt = sb.tile([C, O], F32, tag="wdt")
            bs = sb.tile([1, O], F32, tag="bs")
            nc.sync.dma_start(wnt[:, :], win[f'wnt{li}'][:, :])
            nc.sync.dma_start(wdt[:, :], win[f'wdt{li}'][:, :])
            nc.sync.dma_start(bs[:, :], win[f'bs{li}'][:, :])
            wntr = sb.tile([C, O], F32R, tag="wntr")
            nc.scalar.activation(wntr[:, :], wnt[:, :], AF.Copy)
            wdtr = sb.tile([C, O], F32R, tag="wdtr")
            nc.scalar.activation(wdtr[:, :], wdt[:, :], AF.Copy)

            xsq = sb.tile([C, N], F32, tag="xsq", bufs=1)
            nc.scalar.activation(xsq[:, :], xT[:, :], AF.Square)
            x2dr = sb.tile([C, N], F32R, tag="x2dr", bufs=1)
            nc.scalar.activation(x2dr[:, :], xT[:, :], AF.Copy, bias=0.0, scale=2.0)

            negSr = mp.tile([1, N], F32R, tag="negSr")
            for h in range(2):
                cols = slice(h * 512, (h + 1) * 512)
                S_ps = pss.tile([1, 512], F32, tag="a")
                nc.tensor.matmul(
                    S_ps[:, :], lhsT=onescol[0:C, :], rhs=xsq[:, cols],
                    start=True, stop=True, skip_group_check=True)
                nc.scalar.activation(
                    negSr[0:1, cols], S_ps[:, :], AF.Copy, bias=0.0, scale=-1.0)

            # a-rows to DRAM first so gathers can start as soon as idx ready
            for t in range(NT):
                a_ps = pss.tile([128, O], F32, tag="a")
                nc.tensor.matmul(
                    a_ps[:, :], lhsT=xr[:, t * 128:(t + 1) * 128], rhs=wntr[:, :],
                    start=True, stop=True, skip_group_check=True)
                a_sb = sb.tile([128, O], adt, tag="a_sb")
                nc.scalar.activation(a_sb[:, :], a_ps[:, :], AF.Copy)
                nc.sync.dma_start(a_d[t * 128:(t + 1) * 128, :], a_sb[:, :])

            if pre_tiles is not None:
                pre_tiles()

            m = mp.tile([128, NT, O], F32, tag="m")
            jwap = jw_d[:, :]
            pending_g = []
            pxs = []
            for ot, (opT, opr, orow) in enumerate(out_parts):
                px = pxp.tile([orow, N], F32, tag="px")
                pxs.append(px)
                for h in range(2):
                    cols = slice(h * 512, (h + 1) * 512)
                    nc.tensor.matmul(
                        px[:, cols], lhsT=wdtr[:, ot * 128:ot * 128 + orow],
                        rhs=xr[:, cols],
                        start=True, stop=False, skip_group_check=True)
                    nc.tensor.matmul(
                        px[:, cols], lhsT=bs[0:1, ot * 128:ot * 128 + orow],
                        rhs=onesrow[0:1, cols],
                        start=False, stop=False, skip_group_check=True)

            def reduce_one(tp, g):
                gap = g[:, :, :]
                red_in = bass.AP(
                    gap.tensor, gap.offset,
                    [gap.ap[0], [1, O], [O, KNN]])
                nc.vector.tensor_reduce(
                    out=m[:, tp, :], in_=red_in, axis=AX.X, op=ALU.max)
                for ot, (opT, opr, orow) in enumerate(out_parts):
                    nc.tensor.matmul(
                        pxs[ot][:, tp * 128:(tp + 1) * 128],
                        lhsT=m[:, tp, ot * 128:ot * 128 + orow],
                        rhs=ident[:, 0:128],
                        is_transpose=True, start=False, stop=(tp % 4 == 3),
                        skip_group_check=True)
                if tp == 3 or tp == 7:
                    cols = slice(0, 512) if tp == 3 else slice(512, 1024)
                    for ot, (opT, opr, orow) in enumerate(out_parts):
                        nc.scalar.activation(
                            _slice_cols(opT, cols),
                            pxs[ot][:, cols], AF.Prelu, alpha=NEG_SLOPE)
                        nc.scalar.activation(
                            _slice_cols(opr, cols), _slice_cols(opT, cols), AF.Copy)

            for t in range(NT):
                tcols = slice(t * 128, (t + 1) * 128)
                kp = psb.tile([128, N], F32, tag="big")
                for h in range(2):
                    cols = slice(h * 512, (h + 1) * 512)
                    nc.tensor.matmul(
                        kp[:, cols], lhsT=xr[:, tcols], rhs=x2dr[:, cols],
                        start=True, stop=False, skip_group_check=True)
                    nc.tensor.matmul(
                        kp[:, cols], lhsT=onesrow_r[0:1, tcols],
                        rhs=negSr[0:1, cols],
                        start=False, stop=True, skip_group_check=True)
                # PSUM -> SBUF rounded to f32r: zeroes low mantissa bits
                kb = keyp.tile([128, N], F32R, tag="keysP")
                nc.scalar.activation(kb[:, :], kp[:, :], AF.Copy)
                kbu = kb[:, :].bitcast(U32)
                nc.vector.tensor_tensor(
                    out=kbu, in0=kbu, in1=iotaJ[:, :], op=ALU.bitwise_or)
                kbf = kb[:, :].bitcast(F32)
                v24 = sb.tile([128, 24], F32, tag="v24")
                nc.vector.max(v24[:, 0:8], kbf)
                nc.vector.match_replace(kbf, v24[:, 0:8], kbf, NEG_BIG)
                nc.vector.max(v24[:, 8:16], kbf)
                nc.vector.match_replace(kbf, v24[:, 8:16], kbf, NEG_BIG)
                nc.vector.max(v24[:, 16:24], kbf)
                j20 = sb.tile([128, KNN], U32, tag="j20")
                nc.vector.tensor_scalar(
                    j20[:, :], v24[:, 0:KNN].bitcast(U32), 0x3FF, None,
                    op0=ALU.bitwise_and)
                jf = sb.tile([128, KNN], F32, tag="jf")
                nc.vector.tensor_copy(jf[:, :], j20[:, :])

                # transpose j -> [20, 128], int16, wrapped DRAM round-trip
                jT_ps = pss.tile([KNN, 128], F32, tag="a")
                nc.tensor.matmul(
                    jT_ps[:, :], lhsT=jf[:, :], rhs=ident[:, 0:128],
                    is_transpose=True, start=True, stop=True,
                    skip_group_check=True)
                jTi = sb.tile([KNN, 128], I16, tag="jTi")
                nc.vector.tensor_copy(jTi[:, :], jT_ps[:, :])
                dst = bass.AP(jwap.tensor, jwap.offset + t * 160 * 128,
                              [[1024, KNN], [128, 8], [1, 16]])
                nc.sync.dma_start(
                    dst, jTi[:, :].rearrange("k (h s) -> k h s", s=16))
                src_ap = bass.AP(jwap.tensor, jwap.offset + t * 160 * 128,
                                 [[128, 160], [1, 128]])
                idq = keyp.tile([128, 160], I16, tag="idxq", bufs=4)
                nc.sync.dma_start_transpose(idq[:, :], src_ap)
                for half in (16, 32, 64):
                    nc.sync.dma_start(
                        idq[half:2 * half, :], idq[0:half, :])

                g = gath.tile([128, KNN, O], adt, tag="g", bufs=6)
                nc.gpsimd.dma_gather(
                    out_ap=g[:, :, :], in_ap=a_d[:, :],
                    idxs_ap=idq[:, :],
                    num_idxs=KNN * 128, num_idxs_reg=KNN * 128, elem_size=O,
                    single_packet=False, queue_num=t % 4)
                pending_g.append((t, g))
                if t >= 5:
                    reduce_one(*pending_g.pop(0))

            for tp, g in pending_g:
                reduce_one(tp, g)

            # Prelu / xr copies are emitted per-bank inside reduce_one

        w5r = {}
        for ci, (rows, k0) in enumerate([(64, 0), (64, 64), (128, 128),
                                         (128, 256), (128, 384)]):
            w5c = consts.tile([rows, 512], F32, tag=f"w5c{ci}")
            nc.sync.dma_start(w5c[:, :], w5t_d[k0:k0 + rows, :])
            w5cr = consts.tile([rows, 512], F32R, tag=f"w5cr{ci}")
            nc.scalar.activation(w5cr[:, :], w5c[:, :], AF.Copy)
            w5r[ci] = w5cr
        b5sb = consts.tile([1, 512], F32, tag="b5sb")
        nc.sync.dma_start(b5sb[:, :], b5_d[:, :])
        zpart = persist.tile([128, NT, 512], F16, tag="zpart")
        identr = consts.tile([128, 128], F16, tag="identr")
        nc.scalar.activation(identr[:, :], ident[:, :], AF.Copy)

        def zpart_fill():
            for t in range(NT):
                tcols = slice(t * 128, (t + 1) * 128)
                zp_ps = pss.tile([128, 512], F32, tag="a")
                for ci, (xt, rows) in enumerate(
                        [(x1r, 64), (x2r, 64), (x3r, 128)]):
                    nc.tensor.matmul(
                        zp_ps[:, :], lhsT=xt[:, tcols], rhs=w5r[ci][:, :],
                        start=(ci == 0), stop=(ci == 2), skip_group_check=True)
                nc.scalar.activation(zpart[:, t, :], zp_ps[:, :], AF.Copy)

        edge_layer(1, x0T, x0r, 3, 64, [(x1T[:, :], x1r[:, :], 64)])
        edge_layer(2, x1T, x1r, 64, 64, [(x2T[:, :], x2r[:, :], 64)])
        edge_layer(3, x2T, x2r, 64, 128, [(x3T[:, :], x3r[:, :], 128)])
        edge_layer(4, x3T, x3r, 128, 256,
                   [(x4Ta[:, :], x4ra[:, :], 128), (x4Tb[:, :], x4rb[:, :], 128)],
                   pre_tiles=zpart_fill)

        # ---- head: conv5 (x4 chunks; x1-x3 partials precomputed) + max pool ----
        zmax = persist.tile([128, 512], F32, tag="zmax")
        for t in range(NT):
            tcols = slice(t * 128, (t + 1) * 128)
            z_ps = pss.tile([128, 512], F32, tag="a")
            for ci, (xt, rows, k0) in enumerate(
                    [(x4ra, 128, 256), (x4rb, 128, 384)]):
                nc.tensor.matmul(
                    z_ps[:, :], lhsT=xt[:, tcols], rhs=w5r[3 + ci][:, :],
                    start=(ci == 0), stop=False, skip_group_check=True)
            nc.tensor.matmul(
                z_ps[:, :], lhsT=identr[:, :], rhs=zpart[:, t, :],
                start=False, stop=False, skip_group_check=True)
            nc.tensor.matmul(
                z_ps[:, :], lhsT=onesrow[0:1, tcols],
                rhs=b5sb[:, :], start=False, stop=True, skip_group_check=True)
            if t == 0:
                nc.scalar.activation(zmax[:, :], z_ps[:, :], AF.Copy)
            else:
                nc.vector.tensor_tensor(
                    out=zmax[:, :], in0=zmax[:, :], in1=z_ps[:, :], op=ALU.max)
        # transpose zmax chunks and reduce along free dim -> yT [128, 4]
        yT = persist.tile([128, 4], F32, tag="yT")
        for cchunk in range(4):
            zt_ps = pss.tile([128, 128], F32, tag="a")
            nc.tensor.matmul(
                zt_ps[:, :], lhsT=zmax[:, cchunk * 128:(cchunk + 1) * 128],
                rhs=ident[:, 0:128], is_transpose=True, start=True, stop=True,
                skip_group_check=True)
            nc.vector.tensor_reduce(
                out=yT[:, cchunk:cchunk + 1], in_=zt_ps[:, :],
                axis=AX.X, op=ALU.max)
        yTr = persist.tile([128, 4], F32, tag="yTr")
        nc.scalar.activation(yTr[:, :], yT[:, :], AF.Prelu, alpha=NEG_SLOPE)

        # ---- FC head ----
        wfc1sb = consts.tile([128, 4, 256], F32, tag="wfc1sb")
        for c in range(4):
            nc.sync.dma_start(wfc1sb[:, c, :], wfc1_d[c * 128:(c + 1) * 128, :])
        bfc1sb = consts.tile([128, 2], F32, tag="bfc1sb")
        nc.sync.dma_start(bfc1sb[:, :], bfc1_d[:, :])
        wfc2sb = consts.tile([128, 2, 128], F32, tag="wfc2sb")
        for c in range(2):
            nc.sync.dma_start(wfc2sb[:, c, :], wfc2_d[c * 128:(c + 1) * 128, :])
        bfc2sb = consts.tile([128, 1], F32, tag="bfc2sb")
        nc.sync.dma_start(bfc2sb[:, :], bfc2_d[:, :])
        wfc3sb = consts.tile([128, 40], F32, tag="wfc3sb")
        nc.sync.dma_start(wfc3sb[:, :], wfc3_d[:, :])
        bfc3sb = consts.tile([1, 40], F32, tag="bfc3sb")
        nc.sync.dma_start(bfc3sb[:, :], bfc3_d[:, :])

        h1sb = persist.tile([128, 2], F32, tag="h1sb")
        for mt in range(2):
            h1_ps = pss.tile([128, 1], F32, tag="a")
            for c in range(4):
                nc.tensor.matmul(
                    h1_ps[:, :], lhsT=wfc1sb[:, c, mt * 128:(mt + 1) * 128],
                    rhs=yTr[:, c:c + 1],
                    start=(c == 0), stop=(c == 3), skip_group_check=True)
            nc.scalar.activation(
                h1sb[:, mt:mt + 1], h1_ps[:, :], AF.Prelu,
                bias=bfc1sb[:, mt:mt + 1], scale=1.0, alpha=NEG_SLOPE)
        h2sb = persist.tile([128, 1], F32, tag="h2sb")
        h2_ps = pss.tile([128, 1], F32, tag="a")
        for c in range(2):
            nc.tensor.matmul(
                h2_ps[:, :], lhsT=wfc2sb[:, c, :], rhs=h1sb[:, c:c + 1],
                start=(c == 0), stop=(c == 1), skip_group_check=True)
        nc.scalar.activation(
            h2sb[:, :], h2_ps[:, :], AF.Prelu,
            bias=bfc2sb[:, :], scale=1.0, alpha=NEG_SLOPE)

        out_ps = pss.tile([40, 1], F32, tag="a")
        nc.tensor.matmul(
            out_ps[:, :], lhsT=wfc3sb[:, :], rhs=h2sb[:, :],
            start=True, stop=False, skip_group_check=True)
        nc.tensor.matmul(
            out_ps[:, :], lhsT=bfc3sb[:, :], rhs=onescol[0:1, :],
            start=False, stop=True, skip_group_check=True)
        out_sb = persist.tile([40, 1], F32, tag="out_sb")
        nc.scalar.activation(out_sb[:, :], out_ps[:, :], AF.Copy)
        nc.sync.dma_start(out_d[:, :], out_sb[:, :])


# ---------------------------------------------------------------------------
# harness entry point
# ---------------------------------------------------------------------------
_NC_CACHE = {}


def _get_nc():
    if 'nc' not in _NC_CACHE:
        _NC_CACHE['nc'] = build_nc()
    return _NC_CACHE['nc']


def kernel(**inputs):
    """Full-batch EdgeCNN forward. x: (8, 1024, 3) -> (8, 40) float32.

    Pure data parallel: batch element b runs on NeuronCore b.
    """
    from concourse.bass_utils import run_bass_kernel_spmd

    inp = {k: np.asarray(v) for k, v in inputs.items()}
    prep = host_prep(inp)
    nc = _get_nc()
    in_maps = []
    for b in range(8):
        m = {'x': np.ascontiguousarray(inp['x'][b]).astype(np.float32)}
        m.update(prep)
        in_maps.append(m)
    res = run_bass_kernel_spmd(nc, in_maps, core_ids=list(range(8)))
    out = np.stack([res.results[b]['out'].reshape(40) for b in range(8)])
    return out.astype(np.float32)


# revision 28
# speedup vs baseline: 1.0151x; 1.0151x over previous
"""EdgeCNN (DGCNN) Bass/Tile kernel for TRN2 — one batch element per core.

Per edge-conv layer (N=1024 points, K=20 neighbors):
  1. PE fp32r: packed-key matmul  pd[n,j] = 2<xn,xj> - S[j]   (PSUM, fp32)
     (the -S[n] row term is dropped: constant per row, cannot change top-k)
  2. ACT: PSUM -> SBUF copy rounded to f32r (zeroes low mantissa bits)
  3. DVE: OR in column index j -> packed keys; 3x max8 + 2x match_replace
     -> top-20 packed keys; extract j
  4. idx -> DRAM -> read back in dma_gather wrapped layout (partition = n%16)
  5. SWDGE dma_gather (per tile, round-robin over 4 SWDGE queues) of rows of
     a = x @ (g~ Wn)^T; DVE strided reduce_max over k
  6. PE: c-matmul fp32r (c = x @ (g~(Wc-Wn))^T + b) + transpose(m) in PSUM
  7. ACT: leaky-relu (Prelu alpha=0.2) PSUM -> next layer xT
Head: conv5 via K-chunk accumulation (fp32r), global max-pool, 3 FC layers.
"""

import contextlib

import numpy as np

import concourse.bass as bass
import concourse.bacc as bacc
import concourse.mybir as mybir
from concourse.tile import TileContext
from concourse.masks import make_identity

F32 = mybir.dt.float32
F32R = mybir.dt.float32r
U32 = mybir.dt.uint32
I16 = mybir.dt.int16
F16 = mybir.dt.float16
AF = mybir.ActivationFunctionType
ALU = mybir.AluOpType
AX = mybir.AxisListType

N = 1024
KNN = 20
NT = 8
NEG_SLOPE = 0.2
BNI = np.float32(1.0 / np.sqrt(1.0 + 1e-5))
LAYERS = [(3, 64), (64, 64), (64, 128), (128, 256)]
NEG_BIG = -3.0e38


def host_prep(inp):
    """Fold BN scale/bias into weights; transpose for device layout."""
    d = {}
    for li, (C, O) in enumerate(LAYERS, start=1):
        W = inp[f'W{li}'].astype(np.float32)
        g = inp[f'g{li}'].astype(np.float32)
        b = inp[f'b{li}'].astype(np.float32)
        gt = g * BNI
        Wn = W[:, :C]
        Wc = W[:, C:]
        d[f'wnt{li}'] = np.ascontiguousarray((gt[:, None] * Wn).T)          # (C, O)
        d[f'wdt{li}'] = np.ascontiguousarray((gt[:, None] * (Wc - Wn)).T)   # (C, O)
        d[f'bs{li}'] = b.reshape(1, O).copy()
    g5 = inp['g5'].astype(np.float32) * BNI
    d['w5t'] = np.ascontiguousarray((g5[:, None] * inp['W5']).T)            # (512, 512)
    d['b5'] = inp['b5'].reshape(1, 512).astype(np.float32).copy()
    g1 = inp['bng1'].astype(np.float32) * BNI
    d['wfc1'] = np.ascontiguousarray((g1[:, None] * inp['fc1_w']).T)        # (512, 256)
    bf1 = g1 * inp['fc1_b'].astype(np.float32) + inp['bnb1'].astype(np.float32)
    d['bfc1'] = np.ascontiguousarray(bf1.reshape(2, 128).T)                 # (128, 2)
    g2 = inp['bng2'].astype(np.float32) * BNI
    d['wfc2'] = np.ascontiguousarray((g2[:, None] * inp['fc2_w']).T)        # (256, 128)
    bf2 = g2 * inp['fc2_b'].astype(np.float32) + inp['bnb2'].astype(np.float32)
    d['bfc2'] = np.ascontiguousarray(bf2.reshape(128, 1))                   # (128, 1)
    d['wfc3'] = np.ascontiguousarray(inp['fc3_w'].T)                        # (128, 40)
    d['bfc3'] = inp['fc3_b'].reshape(1, 40).astype(np.float32).copy()
    return d


def build_nc():
    nc = bacc.Bacc("TRN2", target_bir_lowering=False, debug=False, num_devices=8,
                   num_swdge_queues=4)
    with TileContext(nc) as tc:
        _trace(nc, tc)
    nc.compile()
    return nc


def _slice_cols(ap, cols):
    return bass.AP(ap.tensor, ap.offset + cols.start,
                   [ap.ap[0], [1, cols.stop - cols.start]])


def _trace(nc, tc):
    with contextlib.ExitStack() as ctx:
        dram = ctx.enter_context(tc.tile_pool(name="dram", bufs=1, space="DRAM"))
        consts = ctx.enter_context(tc.tile_pool(name="consts", bufs=1))
        persist = ctx.enter_context(tc.tile_pool(name="persist", bufs=1))
        sb = ctx.enter_context(tc.tile_pool(name="sb", bufs=2))
        keyp = ctx.enter_context(tc.tile_pool(name="keyp", bufs=2))
        gath = ctx.enter_context(tc.tile_pool(name="gath", bufs=2))
        mp = ctx.enter_context(tc.tile_pool(name="mp", bufs=1))
        psb = ctx.enter_context(tc.tile_pool(name="psb", bufs=1, space="PSUM"))
        pxp = ctx.enter_context(tc.tile_pool(name="pxp", bufs=2, space="PSUM"))
        pss = ctx.enter_context(tc.tile_pool(name="pss", bufs=2, space="PSUM"))

        # ---- DRAM I/O ----
        x_d = dram.tile([N, 3], F32, kind="ExternalInput", uniquify=False, name="x")
        win = {}
        for li, (C, O) in enumerate(LAYERS, start=1):
            win[f'wnt{li}'] = dram.tile([C, O], F32, kind="ExternalInput", uniquify=False, name=f"wnt{li}")
            win[f'wdt{li}'] = dram.tile([C, O], F32, kind="ExternalInput", uniquify=False, name=f"wdt{li}")
            win[f'bs{li}'] = dram.tile([1, O], F32, kind="ExternalInput", uniquify=False, name=f"bs{li}")
        w5t_d = dram.tile([512, 512], F32, kind="ExternalInput", uniquify=False, name="w5t")
        b5_d = dram.tile([1, 512], F32, kind="ExternalInput", uniquify=False, name="b5")
        wfc1_d = dram.tile([512, 256], F32, kind="ExternalInput", uniquify=False, name="wfc1")
        bfc1_d = dram.tile([128, 2], F32, kind="ExternalInput", uniquify=False, name="bfc1")
        wfc2_d = dram.tile([256, 128], F32, kind="ExternalInput", uniquify=False, name="wfc2")
        bfc2_d = dram.tile([128, 1], F32, kind="ExternalInput", uniquify=False, name="bfc2")
        wfc3_d = dram.tile([128, 40], F32, kind="ExternalInput", uniquify=False, name="wfc3")
        bfc3_d = dram.tile([1, 40], F32, kind="ExternalInput", uniquify=False, name="bfc3")
        out_d = dram.tile([40, 1], F32, kind="ExternalOutput", uniquify=False, name="out")

        a_ds = {li: dram.tile([N, O], F16 if li >= 3 else F32, name=f"a_d{li}")
                for li, (C, O) in enumerate(LAYERS, start=1)}
        jw_ds = {li: dram.tile([NT * (KNN * 8), 128], I16, name=f"jw_d{li}")
                 for li in range(1, 5)}

        # ---- consts ----
        iotaJ = consts.tile([128, N], U32, tag="iotaJ")
        nc.gpsimd.iota(iotaJ[:, :], [[1, N]], base=0, channel_multiplier=0)
        ident = consts.tile([128, 128], F32, tag="ident")
        make_identity(nc, ident[:, :])
        onescol = consts.tile([128, 1], F32, tag="onescol")
        nc.vector.memset(onescol[:, :], 1.0)
        onesrow = consts.tile([1, 512], F32, tag="onesrow")
        nc.vector.memset(onesrow[:, :], 1.0)
        onesrow_r = consts.tile([1, 512], F32R, tag="onesrow_r")
        nc.scalar.activation(onesrow_r[:, :], onesrow[:, :], AF.Copy)

        # persistent feature tensors (F32 + F32R copies)
        x0T = persist.tile([3, N], F32, tag="x0T")
        x0r = persist.tile([3, N], F32R, tag="x0r")
        x1T = persist.tile([64, N], F32, tag="x1T")
        x1r = persist.tile([64, N], F32R, tag="x1r")
        x2T = persist.tile([64, N], F32, tag="x2T")
        x2r = persist.tile([64, N], F32R, tag="x2r")
        x3T = persist.tile([128, N], F32, tag="x3T")
        x3r = persist.tile([128, N], F32R, tag="x3r")
        x4Ta = persist.tile([128, N], F32, tag="x4Ta")
        x4ra = persist.tile([128, N], F32R, tag="x4ra")
        x4Tb = persist.tile([128, N], F32, tag="x4Tb")
        x4rb = persist.tile([128, N], F32R, tag="x4rb")

        # load x transposed: x_d is (N, 3) row-major
        xap = x_d[:, :]
        nc.sync.dma_start(
            x0T[:, :], bass.AP(xap.tensor, xap.offset, [[1, 3], [3, N]]))
        nc.scalar.activation(x0r[:, :], x0T[:, :], AF.Copy)

        def edge_layer(li, xT, xr, C, O, out_parts, pre_tiles=None):
            """out_parts: list of (xT_part_ap, xr_part_ap, orow)."""
            a_d = a_ds[li]
            jw_d = jw_ds[li]
            adt = F16 if li >= 3 else F32
            wnt = sb.tile([C, O], F32, tag="wnt")
            wdt = sb.tile([C, O], F32, tag="wdt")
            bs = sb.tile([1, O], F32, tag="bs")
            nc.sync.dma_start(wnt[:, :], win[f'wnt{li}'][:, :])
            nc.sync.dma_start(wdt[:, :], win[f'wdt{li}'][:, :])
            nc.sync.dma_start(bs[:, :], win[f'bs{li}'][:, :])
            wntr = sb.tile([C, O], F32R, tag="wntr")
            nc.scalar.activation(wntr[:, :], wnt[:, :], AF.Copy)
            wdtr = sb.tile([C, O], F32R, tag="wdtr")
            nc.scalar.activation(wdtr[:, :], wdt[:, :], AF.Copy)

            xsq = sb.tile([C, N], F32, tag="xsq", bufs=1)
            nc.scalar.activation(xsq[:, :], xT[:, :], AF.Square)
            x2dr = sb.tile([C, N], F32R, tag="x2dr", bufs=1)
            nc.scalar.activation(x2dr[:, :], xT[:, :], AF.Copy, bias=0.0, scale=2.0)

            negSr = mp.tile([1, N], F32R, tag="negSr")
            for h in range(2):
                cols = slice(h * 512, (h + 1) * 512)
                S_ps = pss.tile([1, 512], F32, tag="a")
                nc.tensor.matmul(
                    S_ps[:, :], lhsT=onescol[0:C, :], rhs=xsq[:, cols],
                    start=True, stop=True, skip_group_check=True)
                nc.scalar.activation(
                    negSr[0:1, cols], S_ps[:, :], AF.Copy, bias=0.0, scale=-1.0)

            # a-rows to DRAM first so gathers can start as soon as idx ready
            for t in range(NT):
                a_ps = pss.tile([128, O], F32, tag="a")
                nc.tensor.matmul(
                    a_ps[:, :], lhsT=xr[:, t * 128:(t + 1) * 128], rhs=wntr[:, :],
                    start=True, stop=True, skip_group_check=True)
                a_sb = sb.tile([128, O], adt, tag="a_sb")
                nc.scalar.activation(a_sb[:, :], a_ps[:, :], AF.Copy)
                nc.sync.dma_start(a_d[t * 128:(t + 1) * 128, :], a_sb[:, :])

            if pre_tiles is not None:
                pre_tiles()

            m = mp.tile([128, NT, O], F32, tag="m")
            jwap = jw_d[:, :]
            pending_g = []
            pxs = []
            for ot, (opT, opr, orow) in enumerate(out_parts):
                px = pxp.tile([orow, N], F32, tag="px")
                pxs.append(px)
                for h in range(2):
                    cols = slice(h * 512, (h + 1) * 512)
                    nc.tensor.matmul(
                        px[:, cols], lhsT=wdtr[:, ot * 128:ot * 128 + orow],
                        rhs=xr[:, cols],
                        start=True, stop=False, skip_group_check=True)
                    nc.tensor.matmul(
                        px[:, cols], lhsT=bs[0:1, ot * 128:ot * 128 + orow],
                        rhs=onesrow[0:1, 0:512],
                        start=False, stop=False, skip_group_check=True)

            def reduce_one(tp, g):
                gap = g[:, :, :]
                red_in = bass.AP(
                    gap.tensor, gap.offset,
                    [gap.ap[0], [1, O], [O, KNN]])
                nc.vector.tensor_reduce(
                    out=m[:, tp, :], in_=red_in, axis=AX.X, op=ALU.max)
                for ot, (opT, opr, orow) in enumerate(out_parts):
                    nc.tensor.matmul(
                        pxs[ot][:, tp * 128:(tp + 1) * 128],
                        lhsT=m[:, tp, ot * 128:ot * 128 + orow],
                        rhs=ident[:, 0:128],
                        is_transpose=True, start=False, stop=(tp % 4 == 3),
                        skip_group_check=True)
                if tp == 3 or tp == 7:
                    cols = slice(0, 512) if tp == 3 else slice(512, 1024)
                    for ot, (opT, opr, orow) in enumerate(out_parts):
                        nc.scalar.activation(
                            _slice_cols(opT, cols),
                            pxs[ot][:, cols], AF.Prelu, alpha=NEG_SLOPE)
                        nc.scalar.activation(
                            _slice_cols(opr, cols), _slice_cols(opT, cols), AF.Copy)

            for t in range(NT):
                tcols = slice(t * 128, (t + 1) * 128)
                kp = psb.tile([128, N], F32, tag="big")
                for h in range(2):
                    cols = slice(h * 512, (h + 1) * 512)
                    nc.tensor.matmul(
                        kp[:, cols], lhsT=xr[:, tcols], rhs=x2dr[:, cols],
                        start=True, stop=False, skip_group_check=True)
                    nc.tensor.matmul(
                        kp[:, cols], lhsT=onesrow_r[0:1, 0:128],
                        rhs=negSr[0:1, cols],
                        start=False, stop=True, skip_group_check=True)
                # PSUM -> SBUF rounded to f32r: zeroes low mantissa bits
                kb = keyp.tile([128, N], F32R, tag="keysP")
                nc.scalar.activation(kb[:, :], kp[:, :], AF.Copy)
                kbu = kb[:, :].bitcast(U32)
                nc.vector.tensor_tensor(
                    out=kbu, in0=kbu, in1=iotaJ[:, :], op=ALU.bitwise_or)
                kbf = kb[:, :].bitcast(F32)
                v24 = sb.tile([128, 24], F32, tag="v24")
                nc.vector.max(v24[:, 0:8], kbf)
                nc.vector.match_replace(kbf, v24[:, 0:8], kbf, NEG_BIG)
                nc.vector.max(v24[:, 8:16], kbf)
                nc.vector.match_replace(kbf, v24[:, 8:16], kbf, NEG_BIG)
                nc.vector.max(v24[:, 16:24], kbf)
                j20 = sb.tile([128, KNN], U32, tag="j20")
                nc.vector.tensor_scalar(
                    j20[:, :], v24[:, 0:KNN].bitcast(U32), 0x3FF, None,
                    op0=ALU.bitwise_and)
                jf = sb.tile([128, KNN], F32, tag="jf")
                nc.vector.tensor_copy(jf[:, :], j20[:, :])

                # transpose j -> [20, 128], int16, wrapped DRAM round-trip
                jT_ps = pss.tile([KNN, 128], F32, tag="a")
                nc.tensor.matmul(
                    jT_ps[:, :], lhsT=jf[:, :], rhs=ident[:, 0:128],
                    is_transpose=True, start=True, stop=True,
                    skip_group_check=True)
                jTi = sb.tile([KNN, 128], I16, tag="jTi")
                nc.vector.tensor_copy(jTi[:, :], jT_ps[:, :])
                dst = bass.AP(jwap.tensor, jwap.offset + t * 160 * 128,
                              [[1024, KNN], [128, 8], [1, 16]])
                nc.sync.dma_start(
                    dst, jTi[:, :].rearrange("k (h s) -> k h s", s=16))
                src_ap = bass.AP(jwap.tensor, jwap.offset + t * 160 * 128,
                                 [[128, 160], [1, 128]])
                idq = keyp.tile([128, 160], I16, tag="idxq", bufs=4)
                nc.sync.dma_start_transpose(idq[:, :], src_ap)
                for half in (16, 32, 64):
                    nc.sync.dma_start(
                        idq[half:2 * half, :], idq[0:half, :])

                g = gath.tile([128, KNN, O], adt, tag="g", bufs=7)
                nc.gpsimd.dma_gather(
                    out_ap=g[:, :, :], in_ap=a_d[:, :],
                    idxs_ap=idq[:, :],
                    num_idxs=KNN * 128, num_idxs_reg=KNN * 128, elem_size=O,
                    single_packet=False, queue_num=t % 4)
                pending_g.append((t, g))
                if t >= 5:
                    reduce_one(*pending_g.pop(0))

            for tp, g in pending_g:
                reduce_one(tp, g)

            # Prelu / xr copies are emitted per-bank inside reduce_one

        w5r = {}
        for ci, (rows, k0) in enumerate([(64, 0), (64, 64), (128, 128),
                                         (128, 256), (128, 384)]):
            w5c = consts.tile([rows, 512], F32, tag=f"w5c{ci}")
            nc.sync.dma_start(w5c[:, :], w5t_d[k0:k0 + rows, :])
            w5cr = consts.tile([rows, 512], F32R, tag=f"w5cr{ci}")
            nc.scalar.activation(w5cr[:, :], w5c[:, :], AF.Copy)
            w5r[ci] = w5cr
        b5sb = consts.tile([1, 512], F32, tag="b5sb")
        nc.sync.dma_start(b5sb[:, :], b5_d[:, :])
        zpart = persist.tile([128, NT, 512], F16, tag="zpart")
        identr = consts.tile([128, 128], F16, tag="identr")
        nc.scalar.activation(identr[:, :], ident[:, :], AF.Copy)

        def zpart_fill():
            for t in range(NT):
                tcols = slice(t * 128, (t + 1) * 128)
                zp_ps = pss.tile([128, 512], F32, tag="a")
                for ci, (xt, rows) in enumerate(
                        [(x1r, 64), (x2r, 64), (x3r, 128)]):
                    nc.tensor.matmul(
                        zp_ps[:, :], lhsT=xt[:, tcols], rhs=w5r[ci][:, :],
                        start=(ci == 0), stop=(ci == 2), skip_group_check=True)
                nc.scalar.activation(zpart[:, t, :], zp_ps[:, :], AF.Copy)

        edge_layer(1, x0T, x0r, 3, 64, [(x1T[:, :], x1r[:, :], 64)])
        edge_layer(2, x1T, x1r, 64, 64, [(x2T[:, :], x2r[:, :], 64)])
        edge_layer(3, x2T, x2r, 64, 128, [(x3T[:, :], x3r[:, :], 128)])
        edge_layer(4, x3T, x3r, 128, 256,
                   [(x4Ta[:, :], x4ra[:, :], 128), (x4Tb[:, :], x4rb[:, :], 128)],
                   pre_tiles=zpart_fill)

        # ---- head: conv5 (x4 chunks; x1-x3 partials precomputed) + max pool ----
        zmax = persist.tile([128, 512], F32, tag="zmax")
        for t in range(NT):
            tcols = slice(t * 128, (t + 1) * 128)
            z_ps = pss.tile([128, 512], F32, tag="a")
            for ci, (xt, rows, k0) in enumerate(
                    [(x4ra, 128, 256), (x4rb, 128, 384)]):
                nc.tensor.matmul(
                    z_ps[:, :], lhsT=xt[:, tcols], rhs=w5r[3 + ci][:, :],
                    start=(ci == 0), stop=False, skip_group_check=True)
            nc.tensor.matmul(
                z_ps[:, :], lhsT=identr[:, :], rhs=zpart[:, t, :],
                start=False, stop=False, skip_group_check=True)
            nc.tensor.matmul(
                z_ps[:, :], lhsT=onesrow[0:1, 0:128],
                rhs=b5sb[:, :], start=False, stop=True, skip_group_check=True)
            if t == 0:
                nc.scalar.activation(zmax[:, :], z_ps[:, :], AF.Copy)
            else:
                nc.vector.tensor_tensor(
                    out=zmax[:, :], in0=zmax[:, :], in1=z_ps[:, :], op=ALU.max)
        # transpose zmax chunks and reduce along free dim -> yT [128, 4]
        yT = persist.tile([128, 4], F32, tag="yT")
        for cchunk in range(4):
            zt_ps = pss.tile([128, 128], F32, tag="a")
            nc.tensor.matmul(
                zt_ps[:, :], lhsT=zmax[:, cchunk * 128:(cchunk + 1) * 128],
                rhs=ident[:, 0:128], is_transpose=True, start=True, stop=True,
                skip_group_check=True)
            nc.vector.tensor_reduce(
                out=yT[:, cchunk:cchunk + 1], in_=zt_ps[:, :],
                axis=AX.X, op=ALU.max)
        yTr = persist.tile([128, 4], F32, tag="yTr")
        nc.scalar.activation(yTr[:, :], yT[:, :], AF.Prelu, alpha=NEG_SLOPE)

        # ---- FC head ----
        wfc1sb = consts.tile([128, 4, 256], F32, tag="wfc1sb")
        for c in range(4):
            nc.sync.dma_start(wfc1sb[:, c, :], wfc1_d[c * 128:(c + 1) * 128, :])
        bfc1sb = consts.tile([128, 2], F32, tag="bfc1sb")
        nc.sync.dma_start(bfc1sb[:, :], bfc1_d[:, :])
        wfc2sb = consts.tile([128, 2, 128], F32, tag="wfc2sb")
        for c in range(2):
            nc.sync.dma_start(wfc2sb[:, c, :], wfc2_d[c * 128:(c + 1) * 128, :])
        bfc2sb = consts.tile([128, 1], F32, tag="bfc2sb")
        nc.sync.dma_start(bfc2sb[:, :], bfc2_d[:, :])
        wfc3sb = consts.tile([128, 40], F32, tag="wfc3sb")
        nc.sync.dma_start(wfc3sb[:, :], wfc3_d[:, :])
        bfc3sb = consts.tile([1, 40], F32, tag="bfc3sb")
        nc.sync.dma_start(bfc3sb[:, :], bfc3_d[:, :])

        h1sb = persist.tile([128, 2], F32, tag="h1sb")
        for mt in range(2):
            h1_ps = pss.tile([128, 1], F32, tag="a")
            for c in range(4):
                nc.tensor.matmul(
                    h1_ps[:, :], lhsT=wfc1sb[:, c, mt * 128:(mt + 1) * 128],
                    rhs=yTr[:, c:c + 1],
                    start=(c == 0), stop=(c == 3), skip_group_check=True)
            nc.scalar.activation(
                h1sb[:, mt:mt + 1], h1_ps[:, :], AF.Prelu,
                bias=bfc1sb[:, mt:mt + 1], scale=1.0, alpha=NEG_SLOPE)
        h2sb = persist.tile([128, 1], F32, tag="h2sb")
        h2_ps = pss.tile([128, 1], F32, tag="a")
        for c in range(2):
            nc.tensor.matmul(
                h2_ps[:, :], lhsT=wfc2sb[:, c, :], rhs=h1sb[:, c:c + 1],
                start=(c == 0), stop=(c == 1), skip_group_check=True)
        nc.scalar.activation(
            h2sb[:, :], h2_ps[:, :], AF.Prelu,
            bias=bfc2sb[:, :], scale=1.0, alpha=NEG_SLOPE)

        out_ps = pss.tile([40, 1], F32, tag="a")
        nc.tensor.matmul(
            out_ps[:, :], lhsT=wfc3sb[:, :], rhs=h2sb[:, :],
            start=True, stop=False, skip_group_check=True)
        nc.tensor.matmul(
            out_ps[:, :], lhsT=bfc3sb[:, :], rhs=onescol[0:1, :],
            start=False, stop=True, skip_group_check=True)
        out_sb = persist.tile([40, 1], F32, tag="out_sb")
        nc.scalar.activation(out_sb[:, :], out_ps[:, :], AF.Copy)
        nc.sync.dma_start(out_d[:, :], out_sb[:, :])


# ---------------------------------------------------------------------------
# harness entry point
# ---------------------------------------------------------------------------
_NC_CACHE = {}


def _get_nc():
    if 'nc' not in _NC_CACHE:
        _NC_CACHE['nc'] = build_nc()
    return _NC_CACHE['nc']


def kernel(**inputs):
    """Full-batch EdgeCNN forward. x: (8, 1024, 3) -> (8, 40) float32.

    Pure data parallel: batch element b runs on NeuronCore b.
    """
    from concourse.bass_utils import run_bass_kernel_spmd

    inp = {k: np.asarray(v) for k, v in inputs.items()}
    prep = host_prep(inp)
    nc = _get_nc()
    in_maps = []
    for b in range(8):
        m = {'x': np.ascontiguousarray(inp['x'][b]).astype(np.float32)}
        m.update(prep)
        in_maps.append(m)
    res = run_bass_kernel_spmd(nc, in_maps, core_ids=list(range(8)))
    out = np.stack([res.results[b]['out'].reshape(40) for b in range(8)])
    return out.astype(np.float32)
